# revision 22
# baseline (speedup 1.0000x reference)
"""Trainium2 Bass kernel for nn_CXNGeneralLayer (GNN message passing).

z = relu(Gi2j @ (xi W_i + b_i) + Adj2j @ (xj1 W_j1 + b_j1)
         + coAdj2j @ (xj1 W_j2 + b_j2) + Gk2j @ (xk W_k + b_k))

Sharding (per the 1D row-parallel hint): output rows (n_j) are split
across 8 NeuronCores; each core streams its [1024, 8192] shard of all
four operator matrices, which dominate the traffic. The stream is the
bottleneck (HBM ~358 GB/s/core), so the shards are converted to bf16 on
the host (64 MB/core instead of 128 MB; quantization error ~2e-3 abs on
an output scale of ~4.8, far under the 2e-2 gate) and pre-transposed to
[8192(t), 1024(j)] blocks so the contraction dim sits on SBUF
partitions. Blocks are packed so each DMA moves a contiguous 1 MB
[128, 4096] tile (4 t-chunks), alternating between the two HWDGE rings.
The small activations h_m = x_m W_m + b_m are replicated to every core
in bf16 stationary-operand layout, so z^T = sum_m h_m^T @ G_m^T
accumulates directly in PSUM with N=512 moving tiles.
"""

import sys

import numpy as np

if "/opt/trn_rl_repo" not in sys.path:
    sys.path.insert(0, "/opt/trn_rl_repo")

N = 8192  # n_i = n_j = n_k
C = 32  # c_in = c_out
N_CORES = 8
JS = N // N_CORES  # 1024 output rows per core
KP = 128  # contraction partition tile
KCH = N // KP  # 64 t-chunks
TCH = 2  # t-chunks per DMA block (512 KB bf16 per dma_start)
NBLK = KCH // TCH  # 16 blocks per matrix
NJH = 2  # j-halves of 512 (PSUM bank limit for f32 output)

_compiled = None


def _build_program():
    import concourse.mybir as mybir
    import concourse.tile as tile
    from concourse import bacc

    f32 = mybir.dt.float32
    bf16 = mybir.dt.bfloat16
    nc = bacc.Bacc("TRN2", target_bir_lowering=False)

    # G^T shard packed in DMA blocks: gt[b, p, c*JS + j] = G^T[TCH*KP*b + KP*c + p, j]
    gts = [
        nc.dram_tensor(f"gt{m}", [NBLK, KP, TCH * JS], bf16, kind="ExternalInput")
        for m in range(4)
    ]
    # h_m in stationary layout: hs[m][p, 32k+c] = h_m[128k+p, c]
    hs = [
        nc.dram_tensor(f"h{m}", [KP, KCH * C], bf16, kind="ExternalInput")
        for m in range(4)
    ]
    f32r = mybir.dt.float32r
    # band-sum selector: sel[p, c] = 1.0 iff p % 32 == c  (see reduction below)
    sel_d = nc.dram_tensor("sel", [KP, C], f32r, kind="ExternalInput")
    out_t = nc.dram_tensor("outT", [C, JS], f32, kind="ExternalOutput")

    with tile.TileContext(nc) as tc:
        with (
            tc.tile_pool(name="cpool", bufs=1) as cpool,
            tc.tile_pool(name="gpool", bufs=32) as gpool,
            tc.tile_pool(name="zpsum", bufs=1, space="PSUM") as zpsum,
        ):
            # All four h tensors ride the HWDGE rings AHEAD of the G
            # stream (no SWDGE contention -> uniform ~2.4us/MB G cadence).
            h_sb = []
            for m in range(4):
                h = cpool.tile([KP, KCH * C], bf16, tag=f"h{m}", name=f"h{m}")
                (nc.sync if m < 2 else nc.scalar).dma_start(h[:], hs[m][:])
                h_sb.append(h)
            sel_sb = cpool.tile([KP, C], f32r, tag="sel", name="sel")
            nc.gpsimd.dma_start(sel_sb[:], sel_d[:])

            # z^T[c, j] += sum_t h_m[t, c] * G_m^T[t, j], streaming G^T in
            # 512 KB blocks alternating between the two HWDGE rings. Each
            # t-chunk k accumulates into PSUM partition band (k%4)*32 of a
            # [128, 512] tile via PE column tiling (tile_position), so
            # matmuls of adjacent chunks land in different 32-column groups
            # of the PE array and run CONCURRENTLY (~2x PE throughput).
            # With that margin the PE keeps up with the stream even while
            # HAM-cold (1.2 GHz), which removes the cold/warm oscillation
            # that otherwise throttles the DMA stream through the tile
            # pool's backpressure. The 4 partial bands are summed on the
            # DVE at the end (3 adds per j-half, ~microsecond).
            zp = [
                zpsum.tile([4 * C, 512], f32, tag=f"zp{jh}", name=f"zp{jh}")
                for jh in range(NJH)
            ]

            # HAM warm-up: ~8 throwaway matmuls on h0 while the first G
            # block is still in flight, so the PE is already at 2.4 GHz
            # (K=8/8) when the real stream begins. Cold matmuls at 1.2 GHz
            # are slower than the DMA cadence and tip the kernel into a
            # cold/warm oscillation otherwise.
            zw = zpsum.tile([C, 512], f32, tag="warm", name="warm")
            for _ in range(16):
                nc.tensor.matmul(
                    zw[:],
                    h_sb[0][:, 0:C],
                    h_sb[0][:, 0:512],
                    start=True,
                    stop=True,
                )

            pos = 0
            for i in range(4 * NBLK):
                dma_eng = nc.sync if pos % 2 == 0 else nc.scalar
                pos += 1
                m, b = divmod(i, NBLK)
                last_block = i == 4 * NBLK - 1
                # [(ci0, nch), ...] sub-pieces of this block, one DMA each
                pieces = (
                    [(0, TCH)] if not last_block else [(ci, 1) for ci in range(TCH)]
                )
                for pi, (ci0, nch) in enumerate(pieces):
                    if pi > 0:
                        dma_eng = nc.sync if pos % 2 == 0 else nc.scalar
                        pos += 1
                    gt = gpool.tile([KP, TCH * JS], bf16, tag="gt")
                    dma_eng.dma_start(
                        gt[:, : nch * JS], gts[m][b][:, ci0 * JS : (ci0 + nch) * JS]
                    )
                    # jh-major; within a jh the chunk matmuls go to
                    # different column groups and overlap on the PE.
                    for jh in range(NJH):
                        for ci in range(nch):
                            k = TCH * b + ci0 + ci
                            band = k % 4
                            first = m == 0 and k < 4
                            last = m == 3 and k >= KCH - 4
                            nc.tensor.matmul(
                                zp[jh][C * band : C * (band + 1), :],
                                h_sb[m][:, C * k : C * (k + 1)],
                                gt[:, JS * ci + 512 * jh : JS * ci + 512 * (jh + 1)],
                                start=first,
                                stop=last,
                                tile_position=(0, C * band),
                            )

            # Band reduction: drain the [128, 512] PSUM tile to SBUF,
            # then one PE matmul with the 0/1 selector as stationary sums
            # the 4 partition bands (cross-partition add, which DVE/ACT
            # cannot do). relu + store per j-half.
            zsb = cpool.tile([C, JS], f32, tag="zsb")
            for jh in range(NJH):
                drain = cpool.tile([KP, 512], f32r, tag=f"drain{jh}")
                nc.scalar.copy(drain[:], zp[jh][:])
                zs = zpsum.tile([C, 512], f32, tag=f"zs{jh}", name=f"zs{jh}")
                nc.tensor.matmul(zs[:], sel_sb[:], drain[:], start=True, stop=True)
                nc.scalar.activation(
                    zsb[:, 512 * jh : 512 * (jh + 1)],
                    zs[:],
                    mybir.ActivationFunctionType.Relu,
                )
                (nc.sync if jh == 0 else nc.scalar).dma_start(
                    out_t[:, 512 * jh : 512 * (jh + 1)],
                    zsb[:, 512 * jh : 512 * (jh + 1)],
                )

    nc.compile()
    return nc


def _get_program():
    global _compiled
    if _compiled is None:
        _compiled = _build_program()
    return _compiled


def _prep_inputs(inputs):
    """Host-side sharding: returns per-core input maps."""
    import ml_dtypes

    bf16 = ml_dtypes.bfloat16
    f32 = np.float32
    branches = [
        ("Gi2j", "xi", "W_i", "b_i"),
        ("Adj2j", "xj1", "W_j1", "b_j1"),
        ("coAdj2j", "xj1", "W_j2", "b_j2"),
        ("Gk2j", "xk", "W_k", "b_k"),
    ]
    shared = {}
    for m, (_, xn, wn, bn) in enumerate(branches):
        x = np.asarray(inputs[xn], dtype=f32)
        w = np.asarray(inputs[wn], dtype=f32)
        b = np.asarray(inputs[bn], dtype=f32)
        h = x @ w + b  # [N, C] replicated activation, broadcast to all cores
        shared[f"h{m}"] = np.ascontiguousarray(
            h.reshape(KCH, KP, C).transpose(1, 0, 2).reshape(KP, KCH * C)
        ).astype(bf16)

    p = np.arange(KP)
    shared["sel"] = (np.equal.outer(p % C, np.arange(C))).astype(f32)

    in_maps = []
    for s in range(N_CORES):
        im = dict(shared)
        for m, (gn, _, _, _) in enumerate(branches):
            g = np.asarray(inputs[gn])
            blk = g[s * JS : (s + 1) * JS, :].astype(bf16)  # [JS, N]
            # want gt[b, p, ci*JS + j] = blk[j, TCH*KP*b + KP*ci + p]
            gt = (
                blk.reshape(JS, NBLK, TCH, KP)
                .transpose(1, 3, 2, 0)
                .reshape(NBLK, KP, TCH * JS)
            )
            im[f"gt{m}"] = np.ascontiguousarray(gt)
        in_maps.append(im)
    return in_maps


def _spot_check(inputs, out):
    """Cheap host-side validation: recompute a few output rows exactly and
    compare. Catches rare transient device corruption (seen once: first
    execution after a cold NEFF load returned garbage)."""
    f32 = np.float32
    rng = np.random.default_rng(0)
    rows = np.concatenate(
        [s * JS + rng.integers(0, JS, size=2) for s in range(N_CORES)]
    )
    z = np.zeros((len(rows), C), dtype=f32)
    for gn, xn, wn, bn in [
        ("Gi2j", "xi", "W_i", "b_i"),
        ("Adj2j", "xj1", "W_j1", "b_j1"),
        ("coAdj2j", "xj1", "W_j2", "b_j2"),
        ("Gk2j", "xk", "W_k", "b_k"),
    ]:
        x = np.asarray(inputs[xn], dtype=f32)
        h = x @ np.asarray(inputs[wn], dtype=f32) + np.asarray(inputs[bn], dtype=f32)
        z += np.asarray(inputs[gn], dtype=f32)[rows] @ h
    z = np.maximum(z, 0.0)
    err = np.abs(out[rows] - z).max()
    return err <= 0.015 * max(np.abs(z).max(), 1e-6)


def _run(inputs, trace=False):
    from concourse.bass_utils import run_bass_kernel_spmd

    nc = _get_program()
    in_maps = _prep_inputs(inputs)
    res = None
    for attempt in range(3):
        try:
            res = run_bass_kernel_spmd(nc, in_maps, list(range(N_CORES)), trace=trace)
        except Exception:
            # transient device errors (e.g. NRT_EXEC_UNIT_UNRECOVERABLE)
            # clear on re-dispatch
            if attempt == 2:
                raise
            continue
        out = np.concatenate(
            [res.results[s]["outT"] for s in range(N_CORES)], axis=1
        ).T
        out = np.ascontiguousarray(out, dtype=np.float32)
        if _spot_check(inputs, out):
            return out, res
    return out, res


def kernel(**inputs):
    out, _ = _run(inputs, trace=False)
    return out
